# revision 1
# baseline (speedup 1.0000x reference)
"""Trainium2 Bass kernel for BasicSelfAttention2D (spatial-reduction attention).

Reference computation (per image):
    q   = (wq @ x_flat)              [d=32, N=4096]
    xkv = avgpool2x2(x)              [C, Nk=1024]
    k   = wk @ xkv                   [d, Nk]
    v   = wv @ xkv                   [C, Nk]
    attn= softmax(q^T k / sqrt(d))   [N, Nk]
    out = v @ attn^T                 [C, N]
    y   = x + gamma * (wo @ out)

Sharding: data-parallel over batch, one image per NeuronCore (8 cores).

Kernel strategy (per core):
  - scores are built TRANSPOSED  s_T[m, n]  (m = kv position on partitions)
    so that exp(s_T) can directly feed the attention-aggregation matmul
    (which contracts over m on the partition dim).  The softmax denominator
    rsum[n] = sum_m exp(s_T[m,n]) is computed with a DVE add-tree +
    GPSIMD partition_all_reduce, and its reciprocal is applied as a
    per-column scale where it commutes with the out-projection
    (folded into the PSUM->SBUF copy of out_u, before out-proj).
  - q/k projections are emitted 2x column-packed (tile_position) so q,k are
    REPLICATED across two 32-partition groups; score matmuls (K=32) are then
    2-way row-packed, and the next super's score packs are interleaved with
    the current super's aggregation matmuls to keep the in-order PE queue fed.
  - all matmuls are bf16 (1 cyc/row; fp32 is 4x slower and fp32r's fused
    weight-load path only tolerates a single sync-wait in walrus codegen).
    Independent bf16 rounding errors do not amplify through reductions.
    The residual add x + (.) happens in fp32 against fp32 PSUM.
  - host-side prep folds: 1/4 (avgpool mean) into wkT/wvT, gamma into woT,
    and provides bf16 copies of x / weights (layout+dtype prep only).
"""

import ml_dtypes
import numpy as np

import concourse.bacc as bacc
import concourse.mybir as mybir
from concourse.tile import TileContext
from concourse.bass_utils import run_bass_kernel_spmd

B, C, H, W = 8, 256, 64, 64
N = H * W          # 4096
D = 32             # q/k dim
NK = (H // 2) * (W // 2)   # 1024
NCORES = 8

F32 = mybir.dt.float32
F16 = mybir.dt.float16
BF16 = mybir.dt.bfloat16

SCALE = 1.0 / np.sqrt(np.float32(D))   # softmax scale

SUP = 1024          # n-super width (exp granularity)
NSUP = N // SUP     # 4
NCHUNK = 512        # matmul free-dim chunk
MT = NK // 128      # 8 m-tiles


def build_nc():
    nc = bacc.Bacc(None, target_bir_lowering=False, debug=False)

    x_in = nc.dram_tensor("x", [C, N], F32, kind="ExternalInput")
    xb_in = nc.dram_tensor("xb", [C, N], BF16, kind="ExternalInput")
    WPACK = D + D + C + C   # wqT | wkT | wvT | woT  along the free dim
    wall_in = nc.dram_tensor("wall", [C, WPACK], BF16, kind="ExternalInput")
    y_out = nc.dram_tensor("y", [C, N], F32, kind="ExternalOutput")

    with TileContext(nc) as tc:
        with (
            tc.tile_pool(name="big", bufs=1) as big,
            tc.tile_pool(name="work", bufs=2) as work,
            tc.tile_pool(name="etp", bufs=3) as etp,
            tc.tile_pool(name="ystage", bufs=4) as ypool,
            tc.tile_pool(name="xres", bufs=4) as xpool,
            tc.tile_pool(name="rows", bufs=4) as rowp,
            tc.tile_pool(name="ps_sc", bufs=2, space="PSUM") as ps_sc,
            tc.tile_pool(name="ps_ac", bufs=4, space="PSUM") as ps_ac,
        ):
            # ---------------- persistent SBUF ----------------
            xb_sb = big.tile([128, 2, N], BF16, tag="xb")     # c-half major
            xkv_sb = big.tile([128, 2, NK], BF16, tag="xkv")
            qrep_sb = big.tile([128, N], BF16, tag="qrep")    # q replicated 4x
            krep_sb = big.tile([128, NK], BF16, tag="krep")   # k replicated 4x
            vT_sb = big.tile([128, MT, C], BF16, tag="vT")    # v transposed
            # DMA staging for weights; the ACT copy into the real tile makes
            # every matmul weight-dependency an ACT-sem wait (merges with the
            # other ACT waits -- walrus caps matmuls at 2 sync waits).
            w_st = big.tile([128, 2, WPACK], BF16, tag="w_st")
            w_sb = big.tile([128, 2, WPACK], BF16, tag="w_sb")
            wq_sb = w_sb[:, :, 0:D]
            wk_sb = w_sb[:, :, D : 2 * D]
            wv_sb = w_sb[:, :, 2 * D : 2 * D + C]
            wo_sb = w_sb[:, :, 2 * D + C :].rearrange("p t (o k) -> p t o k", o=2)

            # ---------------- input DMAs ----------------
            nc.sync.dma_start(
                out=w_st, in_=wall_in.rearrange("(t p) w -> p t w", p=128)
            )
            for s in range(NSUP):
                nsl = slice(s * SUP, (s + 1) * SUP)
                for ch in range(2):
                    rows = slice(ch * 128, (ch + 1) * 128)
                    nc.sync.dma_start(out=xb_sb[:, ch, nsl], in_=xb_in[rows, nsl])
            nc.scalar.activation(
                out=w_sb, in_=w_st, func=mybir.ActivationFunctionType.Copy
            )
            # ones vectors for the softmax-denominator matmuls
            ones_col = big.tile([128, 1], BF16, tag="ones_col")
            nc.vector.memset(ones_col, 1.0)
            ones_row = big.tile([1, 128], F16, tag="ones_row")
            nc.vector.memset(ones_row, 1.0)
            # dummy exp: pulls the ACT exp table load into the setup phase
            warm = big.tile([128, 1], F32, tag="warm")
            nc.vector.memset(warm, 0.0)
            nc.scalar.activation(
                out=warm, in_=warm, func=mybir.ActivationFunctionType.Exp
            )
            # HAM warm-up: ~8 dummy matmuls fill the PE-idle DMA-wait window
            # so the clock gate is at 8/8 before the first real matmul
            wrm_sb = big.tile([128, NCHUNK], BF16, tag="wrm")
            nc.vector.memset(wrm_sb, 0.0)
            wrm_ps = ps_sc.tile([128, NCHUNK], F32, tag="sc", name="wrm_ps")
            for i in range(8):
                nc.tensor.matmul(
                    wrm_ps, lhsT=wrm_sb[:, 0:128], rhs=wrm_sb,
                    start=(i == 0), stop=(i == 7),
                )

            # ---------------- q projection (replicated 2x col-packed) -----
            for cn in range(N // NCHUNK):
                nsl = slice(cn * NCHUNK, (cn + 1) * NCHUNK)
                qp = ps_sc.tile([128, NCHUNK], F32, tag="sc")
                for j in range(2):
                    for ch in range(2):
                        nc.tensor.matmul(
                            qp[32 * j : 32 * (j + 1), :],
                            lhsT=wq_sb[:, ch, :],
                            rhs=xb_sb[:, ch, nsl],
                            start=(ch == 0),
                            stop=(ch == 1),
                            tile_position=(0, 32 * j),
                        )
                nc.scalar.activation(
                    out=qrep_sb[0:64, nsl], in_=qp[0:64, :],
                    func=mybir.ActivationFunctionType.Copy,
                )

            # ---------------- avgpool (sum; /4 folded into wkT/wvT) -------
            for ch in range(2):
                xw = work.tile([128, 64, 32], BF16, tag="xw")  # w-paired sums
                x4 = xb_sb[:, ch, :].rearrange("p (h w t) -> p h w t", h=64, w=32)
                for s in range(NSUP):
                    hs = slice(s * 16, (s + 1) * 16)
                    nc.vector.tensor_add(
                        out=xw[:, hs, :], in0=x4[:, hs, :, 0], in1=x4[:, hs, :, 1]
                    )
                xh = xw.rearrange("p (h2 t) w -> p h2 t w", t=2)
                xkv_v = xkv_sb[:, ch, :].rearrange("p (a b) -> p a b", a=32)
                for s in range(NSUP):
                    h2s = slice(s * 8, (s + 1) * 8)
                    nc.vector.tensor_add(
                        out=xkv_v[:, h2s, :],
                        in0=xh[:, h2s, 0, :],
                        in1=xh[:, h2s, 1, :],
                    )

            # ---------------- k projection (replicated 2x col-packed) -----
            for cn in range(NK // NCHUNK):
                nsl = slice(cn * NCHUNK, (cn + 1) * NCHUNK)
                kp = ps_sc.tile([128, NCHUNK], F32, tag="sc")
                for j in range(2):
                    for ch in range(2):
                        nc.tensor.matmul(
                            kp[32 * j : 32 * (j + 1), :],
                            lhsT=wk_sb[:, ch, :],
                            rhs=xkv_sb[:, ch, nsl],
                            start=(ch == 0),
                            stop=(ch == 1),
                            tile_position=(0, 32 * j),
                        )
                nc.scalar.activation(
                    out=krep_sb[0:64, nsl], in_=kp[0:64, :],
                    func=mybir.ActivationFunctionType.Copy,
                )

            # ---------------- v projection (transposed) ----------------
            for mt in range(MT):
                msl = slice(mt * 128, (mt + 1) * 128)
                vp = ps_ac.tile([128, C], F32, tag="ac")
                for ch in range(2):
                    nc.tensor.matmul(
                        vp,
                        lhsT=xkv_sb[:, ch, msl],
                        rhs=wv_sb[:, ch, :],
                        start=(ch == 0),
                        stop=(ch == 1),
                    )
                nc.scalar.activation(
                    out=vT_sb[:, mt, :], in_=vp,
                    func=mybir.ActivationFunctionType.Copy,
                )

            # ---------------- main loop over n-supers ----------------
            def make_pack_ops(s):
                """Return 4 closures; each emits one 2-way-packed score pack
                (2 matmuls + 2 exps + 1 stage-1 add) for super s."""
                et = etp.tile([128, MT, SUP], BF16, tag="et", name="et")
                p4 = work.tile([128, 4, SUP], BF16, tag="p4", name="p4")

                def pack(mp):
                    sc_ps = [
                        ps_sc.tile([128, SUP], F32, tag="sc", name=f"sc{i}")
                        for i in range(2)
                    ]
                    for half in range(2):
                        hsl = slice(s * SUP + half * NCHUNK,
                                    s * SUP + (half + 1) * NCHUNK)
                        osl = slice(half * NCHUNK, (half + 1) * NCHUNK)
                        for i in range(2):
                            mt = 2 * mp + i
                            base = slice(32 * i, 32 * (i + 1))
                            nc.tensor.matmul(
                                sc_ps[i][:, osl],
                                lhsT=krep_sb[base, mt * 128 : (mt + 1) * 128],
                                rhs=qrep_sb[base, hsl],
                                tile_position=(32 * i, 0),
                            )
                    for i in range(2):
                        nc.scalar.activation(
                            out=et[:, 2 * mp + i, :], in_=sc_ps[i],
                            func=mybir.ActivationFunctionType.Exp,
                            scale=float(SCALE),
                        )
                    nc.vector.tensor_add(
                        out=p4[:, mp, :], in0=et[:, 2 * mp, :],
                        in1=et[:, 2 * mp + 1, :],
                    )

                return et, p4, [lambda mp=mp: pack(mp) for mp in range(4)]

            cur = make_pack_ops(0)
            for op in cur[2]:
                op()
            for s in range(NSUP):
                et, p4, _ = cur
                nxt_packs = []
                if s + 1 < NSUP:
                    cur = make_pack_ops(s + 1)
                    nxt_packs = list(cur[2])

                def next_pack():
                    if nxt_packs:
                        nxt_packs.pop(0)()

                # denominator tree stages 2+3 -> single partial p1
                p2 = work.tile([128, 2, SUP], BF16, tag="p2")
                nc.vector.tensor_add(out=p2[:, 0, :], in0=p4[:, 0, :], in1=p4[:, 1, :])
                nc.vector.tensor_add(out=p2[:, 1, :], in0=p4[:, 2, :], in1=p4[:, 3, :])
                p1 = work.tile([128, SUP], BF16, tag="p1")
                nc.vector.tensor_add(out=p1, in0=p2[:, 0, :], in1=p2[:, 1, :])

                # denominator finish: ones-matmul row-sum, fp16 row copy,
                # K=1 broadcast matmul, fast reciprocal into SBUF
                outu = work.tile([128, 2, SUP], BF16, tag="outu")
                av_ps = {}
                for c in range(2):
                    for half in range(2):
                        av_ps[c, half] = ps_ac.tile(
                            [128, NCHUNK], F32, tag="ac", name=f"av{c}{half}"
                        )
                scale_sb = {}
                for half in range(2):
                    osl = slice(half * NCHUNK, (half + 1) * NCHUNK)
                    rs_ps = ps_ac.tile([1, NCHUNK], F32, tag="ac", name="rs")
                    nc.tensor.matmul(rs_ps, lhsT=ones_col, rhs=p1[:, osl])
                    rs_row = rowp.tile([1, NCHUNK], F16, tag="rs_row")
                    nc.scalar.activation(
                        out=rs_row, in_=rs_ps,
                        func=mybir.ActivationFunctionType.Copy,
                    )
                    bc_ps = ps_ac.tile([128, NCHUNK], F32, tag="ac", name="bc")
                    nc.tensor.matmul(bc_ps, lhsT=ones_row, rhs=rs_row)
                    sc_t = rowp.tile([128, NCHUNK], F32, tag="scale")
                    nc.vector.reciprocal_approx_fast(out=sc_t, in_=bc_ps)
                    scale_sb[half] = sc_t

                # attention aggregation (contract m), interleaved with the
                # NEXT super's score packs so the PE queue always has ready
                # work while ACT pipelines the exps
                for c in range(2):
                    for mt in range(MT):
                        if mt % 2 == 0:
                            next_pack()
                        for half in range(2):
                            osl = slice(half * NCHUNK, (half + 1) * NCHUNK)
                            nc.tensor.matmul(
                                av_ps[c, half],
                                lhsT=vT_sb[:, mt, c * 128 : (c + 1) * 128],
                                rhs=et[:, mt, osl],
                                start=(mt == 0),
                                stop=(mt == MT - 1),
                            )
                    for half in range(2):
                        osl = slice(half * NCHUNK, (half + 1) * NCHUNK)
                        nc.vector.scalar_tensor_tensor(
                            out=outu[:, c, osl],
                            in0=av_ps[c, half],
                            scalar=1.0,
                            in1=scale_sb[half],
                            op0=mybir.AluOpType.mult,
                            op1=mybir.AluOpType.mult,
                        )
                while nxt_packs:
                    next_pack()

                # out-projection + residual add (fp32) + store
                for half in range(2):
                    osl = slice(half * NCHUNK, (half + 1) * NCHUNK)
                    fsl = slice(s * SUP + half * NCHUNK,
                                s * SUP + (half + 1) * NCHUNK)
                    for ot in range(2):
                        xres = xpool.tile([128, NCHUNK], F32, tag="xr")
                        nc.gpsimd.dma_start(
                            out=xres, in_=x_in[ot * 128 : (ot + 1) * 128, fsl]
                        )
                        op_ps = ps_ac.tile([128, NCHUNK], F32, tag="ac", name="op")
                        for ch in range(2):
                            nc.tensor.matmul(
                                op_ps,
                                lhsT=wo_sb[:, ch, ot, :],
                                rhs=outu[:, ch, osl],
                                start=(ch == 0),
                                stop=(ch == 1),
                            )
                        y_st = ypool.tile([128, NCHUNK], F32, tag="y")
                        nc.vector.tensor_add(out=y_st, in0=xres, in1=op_ps)
                        nc.sync.dma_start(
                            out=y_out[ot * 128 : (ot + 1) * 128, fsl], in_=y_st
                        )
    nc.compile()
    return nc


_NC_CACHE = {}


def _get_nc():
    if "nc" not in _NC_CACHE:
        _NC_CACHE["nc"] = build_nc()
    return _NC_CACHE["nc"]


def _prep_inputs(x, wq, wk, wv, wo, gamma):
    bf = ml_dtypes.bfloat16
    x = np.ascontiguousarray(np.asarray(x, dtype=np.float32))
    xb = x.astype(bf)
    wqT = np.asarray(wq, np.float32).T
    wkT = np.asarray(wk, np.float32).T * 0.25
    wvT = np.asarray(wv, np.float32).T * 0.25
    woT = np.float32(np.asarray(gamma, np.float32)[0]) * np.asarray(wo, np.float32).T
    wall = np.ascontiguousarray(
        np.concatenate([wqT, wkT, wvT, woT], axis=1)
    ).astype(bf)
    in_maps = []
    for i in range(NCORES):
        in_maps.append({
            "x": np.ascontiguousarray(x[i].reshape(C, N)),
            "xb": np.ascontiguousarray(xb[i].reshape(C, N)),
            "wall": wall,
        })
    return in_maps


def run(x, wq, wk, wv, wo, gamma, trace=False, **trace_kwargs):
    nc = _get_nc()
    in_maps = _prep_inputs(x, wq, wk, wv, wo, gamma)
    res = run_bass_kernel_spmd(
        nc, in_maps, list(range(NCORES)), trace=trace, **trace_kwargs
    )
    y = np.stack([res.results[i]["y"].reshape(C, H, W) for i in range(NCORES)])
    return y, res


def kernel(x, wq, wk, wv, wo, gamma):
    y, _ = run(x, wq, wk, wv, wo, gamma, trace=False)
    return y



# revision 8
# speedup vs baseline: 1.1704x; 1.1704x over previous
"""Trainium2 Bass kernel for BasicSelfAttention2D (spatial-reduction attention).

Reference computation (per image):
    q   = (wq @ x_flat)              [d=32, N=4096]
    xkv = avgpool2x2(x)              [C, Nk=1024]
    k   = wk @ xkv                   [d, Nk]
    v   = wv @ xkv                   [C, Nk]
    attn= softmax(q^T k / sqrt(d))   [N, Nk]
    out = v @ attn^T                 [C, N]
    y   = x + gamma * (wo @ out)

Sharding: data-parallel over batch, one image per NeuronCore (8 cores).

Kernel strategy (per core):
  - HOST folds wov = 0.25*gamma*(wo @ wv): the aggregation matmul then
    directly produces the final (pre-residual) output -- no separate
    out-projection pass on the PE.
  - TRANSPOSED aggregation: out_T[n, c] = sum_m et[m, n] * vT[m, c] with
    lhsT = et (m on partitions) and rhs = [vT | ones].  The appended ones
    column makes column C of the PSUM output the softmax denominator
    rsum[n], which lands per-partition -- so softmax scale + residual add
    fuse into ONE scalar_tensor_tensor per 128-row n-tile:
        y_T[n, :] = av[n, :] * recip(rsum[n]) + x_T[n, :]
    x is loaded (and y stored) in transposed [N, C] layout; the host
    transposes y back (cheap numpy work, not HW time).
  - scores are built transposed s_T[m, n] with 4-way row-packed K=32
    matmuls (tile_position=(32i, 0)); q/k are replicated 4x across
    partition groups by 4-way column-packed projections.  exp runs on ACT
    over fused [128, 4, 512] PSUM tiles (one instr per 4 m-tiles).
  - all matmuls bf16; residual add in fp32 against fp32 PSUM.
"""

import ml_dtypes
import numpy as np

import concourse.bacc as bacc
import concourse.mybir as mybir
from concourse.tile import TileContext
from concourse.bass_utils import run_bass_kernel_spmd

B, C, H, W = 8, 256, 64, 64
N = H * W          # 4096
D = 32             # q/k dim
NK = (H // 2) * (W // 2)   # 1024
NCORES = 8

F32 = mybir.dt.float32
BF16 = mybir.dt.bfloat16

SCALE = 1.0 / np.sqrt(np.float32(D))   # softmax scale

SUP = 1024          # n-super width
NSUP = N // SUP     # 4
MT = NK // 128      # 8 m-tiles
VW = 257            # aggregation rhs width: C channels + ones column


def build_nc():
    nc = bacc.Bacc(None, target_bir_lowering=False, debug=False)

    xb_in = nc.dram_tensor("xb", [C, N], BF16, kind="ExternalInput")
    xt_in = nc.dram_tensor("xt", [N, C], F32, kind="ExternalInput")
    WPACK = D + D + C   # wqT | wkT | wovT along the free dim
    wall_in = nc.dram_tensor("wall", [C, WPACK], BF16, kind="ExternalInput")
    y_out = nc.dram_tensor("y", [N, C], F32, kind="ExternalOutput")
    ytv = y_out.rearrange("(s t p) c -> p s t c", p=128, t=MT)

    with TileContext(nc) as tc:
        with (
            tc.tile_pool(name="big", bufs=1) as big,
            tc.tile_pool(name="etp", bufs=2) as etp,
            tc.tile_pool(name="xres", bufs=2) as xpool,
            tc.tile_pool(name="ystage", bufs=2) as ypool,
            tc.tile_pool(name="small", bufs=8) as smallp,
            tc.tile_pool(name="ps_sc", bufs=1, space="PSUM") as ps_sc,
            tc.tile_pool(name="ps_av", bufs=4, space="PSUM") as ps_av,
        ):
            # ---------------- persistent SBUF ----------------
            xb_sb = big.tile([128, 2, N], BF16, tag="xb")     # c-half major
            xkv_sb = big.tile([128, 2, NK], BF16, tag="xkv")
            q4_sb = big.tile([128, N], BF16, tag="q4")        # q replicated 4x
            k4_sb = big.tile([128, NK], BF16, tag="k4")       # k replicated 4x
            vT_sb = big.tile([128, MT, VW], BF16, tag="vT")   # [v^T | ones]
            # DMA staging for weights; the ACT copy into the real tile makes
            # every matmul weight-dependency an ACT-sem wait.
            w_st = big.tile([128, 2, WPACK], BF16, tag="w_st")
            w_sb = big.tile([128, 2, WPACK], BF16, tag="w_sb")
            wq_sb = w_sb[:, :, 0:D]
            wk_sb = w_sb[:, :, D : 2 * D]
            wv_sb = w_sb[:, :, 2 * D :]

            # ---------------- input DMAs ----------------
            nc.sync.dma_start(
                out=w_st, in_=wall_in.rearrange("(t p) w -> p t w", p=128)
            )
            xbv = xb_in.rearrange("(t p) n -> p t n", p=128)
            for s in range(NSUP):
                nsl = slice(s * SUP, (s + 1) * SUP)
                nc.sync.dma_start(out=xb_sb[:, :, nsl], in_=xbv[:, :, nsl])
            xtv = xt_in.rearrange("(s t p) c -> p s t c", p=128, t=MT)
            nc.scalar.activation(
                out=w_sb, in_=w_st, func=mybir.ActivationFunctionType.Copy
            )
            # ones column of the aggregation rhs
            nc.vector.memset(vT_sb[:, :, C : C + 1], 1.0)
            # dummy exp: pulls the ACT exp table load into the setup phase
            warm = smallp.tile([128, 1], F32, tag="warm")
            nc.vector.memset(warm, 0.0)
            nc.scalar.activation(
                out=warm, in_=warm, func=mybir.ActivationFunctionType.Exp
            )
            # HAM warm-up: dummy matmuls fill the PE-idle DMA-wait window
            wrm_sb = smallp.tile([128, 512], BF16, tag="wrm")
            nc.vector.memset(wrm_sb, 0.0)
            wrm_ps = ps_sc.tile([128, 512], F32, tag="sc", name="wrm_ps")
            for i in range(8):
                nc.tensor.matmul(
                    wrm_ps, lhsT=wrm_sb[:, 0:128], rhs=wrm_sb,
                    start=(i == 0), stop=(i == 7),
                )

            # -------- prologue: per-super q-proj / avgpool / k / v ---------
            for s in range(NSUP):
                nsl = slice(s * SUP, (s + 1) * SUP)
                # q projection, 4x column-packed (replicated on 4 groups)
                qp = ps_sc.tile([128, SUP], F32, tag="sc", name="qp")
                for j in range(4):
                    for half in range(2):
                        hsl = slice(half * 512, (half + 1) * 512)
                        for ch in range(2):
                            nc.tensor.matmul(
                                qp[32 * j : 32 * (j + 1), hsl],
                                lhsT=wq_sb[:, ch, :],
                                rhs=xb_sb[:, ch, s * SUP + half * 512 :
                                          s * SUP + (half + 1) * 512],
                                start=(ch == 0),
                                stop=(ch == 1),
                                tile_position=(0, 32 * j),
                            )
                nc.scalar.activation(
                    out=q4_sb[:, nsl], in_=qp,
                    func=mybir.ActivationFunctionType.Copy,
                )

                # avgpool (sum; /4 folded into wkT/wovT on host)
                x4 = xb_sb[:, :, nsl].rearrange(
                    "p c (h w t) -> p c h w t", h=16, w=32
                )
                for ch in range(2):
                    xw = smallp.tile([128, 16, 32], BF16, tag="xw")
                    nc.vector.tensor_add(
                        out=xw, in0=x4[:, ch, :, :, 0], in1=x4[:, ch, :, :, 1]
                    )
                    xh = xw.rearrange("p (h2 t) w -> p h2 t w", t=2)
                    xkv_v = xkv_sb[:, ch, s * 256 : (s + 1) * 256].rearrange(
                        "p (a b) -> p a b", a=8
                    )
                    nc.vector.tensor_add(
                        out=xkv_v, in0=xh[:, :, 0, :], in1=xh[:, :, 1, :]
                    )

                # k projection for this m-chunk, 4x column-packed
                msl = slice(s * 256, (s + 1) * 256)
                kp = ps_av.tile([128, 256], F32, tag="av", name="kp")
                for j in range(4):
                    for ch in range(2):
                        nc.tensor.matmul(
                            kp[32 * j : 32 * (j + 1), :],
                            lhsT=wk_sb[:, ch, :],
                            rhs=xkv_sb[:, ch, msl],
                            start=(ch == 0),
                            stop=(ch == 1),
                            tile_position=(0, 32 * j),
                        )
                nc.vector.tensor_copy(out=k4_sb[:, msl], in_=kp)

                # v projection (transposed, wov folded) for 2 m-tiles
                for mt in (2 * s, 2 * s + 1):
                    vp = ps_av.tile([128, C], F32, tag="av", name="vp")
                    for ch in range(2):
                        nc.tensor.matmul(
                            vp,
                            lhsT=xkv_sb[:, ch, mt * 128 : (mt + 1) * 128],
                            rhs=wv_sb[:, ch, :],
                            start=(ch == 0),
                            stop=(ch == 1),
                        )
                    nc.vector.tensor_copy(out=vT_sb[:, mt, 0:C], in_=vp)

            # residual prefetch for super 0 -- on the sync queue so it
            # issues AFTER the xb loads (FIFO per queue) and doesn't steal
            # HBM bandwidth from the prologue-critical xb DMAs
            xt_t = {0: xpool.tile([128, MT, C], F32, tag="xt", name="xt0")}
            nc.sync.dma_start(out=xt_t[0], in_=xtv[:, 0])

            # ---------------- main loop over n-supers ----------------
            def make_packs(s):
                """4 closures; each emits one 4-way row-packed score pack
                (4 matmuls over m-tiles of one group + 1 fused exp)."""
                et = etp.tile([128, MT, SUP], BF16, tag="et", name="et")

                def pack(g):
                    # g in 0..3: (m-group, n-half) = (g//2, g%2)
                    mg, half = g // 2, g % 2
                    hsl = slice(s * SUP + half * 512, s * SUP + (half + 1) * 512)
                    sc = ps_sc.tile([128, 4, 512], F32, tag="sc", name="sc")
                    for i in range(4):
                        mt = 4 * mg + i
                        base = slice(32 * i, 32 * (i + 1))
                        nc.tensor.matmul(
                            sc[:, i, :],
                            lhsT=k4_sb[base, mt * 128 : (mt + 1) * 128],
                            rhs=q4_sb[base, hsl],
                            tile_position=(32 * i, 0),
                        )
                    nc.scalar.activation(
                        out=et[:, 4 * mg : 4 * mg + 4,
                               half * 512 : (half + 1) * 512],
                        in_=sc,
                        func=mybir.ActivationFunctionType.Exp,
                        scale=float(SCALE),
                    )

                return et, [lambda g=g: pack(g) for g in range(4)]

            cur = make_packs(0)
            for op in cur[1]:
                op()
            for s in range(NSUP):
                et, _ = cur
                nxt_packs = []
                if s + 1 < NSUP:
                    cur = make_packs(s + 1)
                    nxt_packs = list(cur[1])
                    xt_t[s + 1] = xpool.tile(
                        [128, MT, C], F32, tag="xt", name="xtn"
                    )
                    nc.gpsimd.dma_start(out=xt_t[s + 1], in_=xtv[:, s + 1])

                def next_pack():
                    if nxt_packs:
                        nxt_packs.pop(0)()

                y_sb = ypool.tile([128, MT, C], F32, tag="y")
                for tg in range(2):
                    av = {}
                    for mt in range(MT):
                        if mt in (1, 5):
                            next_pack()
                        for t in range(4 * tg, 4 * tg + 4):
                            if mt == 0:
                                av[t] = ps_av.tile(
                                    [128, VW], F32, tag="av", name=f"av{t % 4}"
                                )
                            nc.tensor.matmul(
                                av[t],
                                lhsT=et[:, mt, t * 128 : (t + 1) * 128],
                                rhs=vT_sb[:, mt, :],
                                start=(mt == 0),
                                stop=(mt == MT - 1),
                            )
                    for t in range(4 * tg, 4 * tg + 4):
                        rc = smallp.tile([128, 1], F32, tag="rc")
                        nc.vector.reciprocal(out=rc, in_=av[t][:, C : C + 1])
                        nc.vector.scalar_tensor_tensor(
                            out=y_sb[:, t, :],
                            in0=av[t][:, 0:C],
                            scalar=rc,
                            in1=xt_t[s][:, t, :],
                            op0=mybir.AluOpType.mult,
                            op1=mybir.AluOpType.add,
                        )
                while nxt_packs:
                    next_pack()
                nc.sync.dma_start(out=ytv[:, s], in_=y_sb)
    nc.compile()
    return nc


_NC_CACHE = {}


def _get_nc():
    if "nc" not in _NC_CACHE:
        _NC_CACHE["nc"] = build_nc()
    return _NC_CACHE["nc"]


def _prep_inputs(x, wq, wk, wv, wo, gamma):
    bf = ml_dtypes.bfloat16
    x = np.asarray(x, dtype=np.float32).reshape(B, C, N)
    g = np.float32(np.asarray(gamma, np.float32)[0])
    wqT = np.asarray(wq, np.float32).T
    wkT = np.asarray(wk, np.float32).T * 0.25
    wov = (0.25 * g) * (np.asarray(wo, np.float32) @ np.asarray(wv, np.float32))
    wall = np.ascontiguousarray(
        np.concatenate([wqT, wkT, wov.T], axis=1)
    ).astype(bf)
    in_maps = []
    for i in range(NCORES):
        xi = x[i]
        in_maps.append({
            "xb": np.ascontiguousarray(xi).astype(bf),
            "xt": np.ascontiguousarray(xi.T),
            "wall": wall,
        })
    return in_maps


def run(x, wq, wk, wv, wo, gamma, trace=False, **trace_kwargs):
    nc = _get_nc()
    in_maps = _prep_inputs(x, wq, wk, wv, wo, gamma)
    res = run_bass_kernel_spmd(
        nc, in_maps, list(range(NCORES)), trace=trace, **trace_kwargs
    )
    y = np.stack([
        np.ascontiguousarray(res.results[i]["y"].T).reshape(C, H, W)
        for i in range(NCORES)
    ])
    return y, res


def kernel(x, wq, wk, wv, wo, gamma):
    y, _ = run(x, wq, wk, wv, wo, gamma, trace=False)
    return y


# revision 9
# speedup vs baseline: 1.1728x; 1.0020x over previous
"""Trainium2 Bass kernel for BasicSelfAttention2D (spatial-reduction attention).

Reference computation (per image):
    q   = (wq @ x_flat)              [d=32, N=4096]
    xkv = avgpool2x2(x)              [C, Nk=1024]
    k   = wk @ xkv                   [d, Nk]
    v   = wv @ xkv                   [C, Nk]
    attn= softmax(q^T k / sqrt(d))   [N, Nk]
    out = v @ attn^T                 [C, N]
    y   = x + gamma * (wo @ out)

Sharding: data-parallel over batch, one image per NeuronCore (8 cores).

Kernel strategy (per core):
  - HOST folds wov = 0.25*gamma*(wo @ wv): the aggregation matmul then
    directly produces the final (pre-residual) output -- no separate
    out-projection pass on the PE.
  - TRANSPOSED aggregation: out_T[n, c] = sum_m et[m, n] * vT[m, c] with
    lhsT = et (m on partitions) and rhs = [vT | ones].  The appended ones
    column makes column C of the PSUM output the softmax denominator
    rsum[n], which lands per-partition -- so softmax scale + residual add
    fuse into ONE scalar_tensor_tensor per 128-row n-tile:
        y_T[n, :] = av[n, :] * recip(rsum[n]) + x_T[n, :]
    x is loaded (and y stored) in transposed [N, C] layout; the host
    transposes y back (cheap numpy work, not HW time).
  - scores are built transposed s_T[m, n]: one pack per m-tile covers both
    512-halves of the n-super concurrently on two distinct 32-row PE
    groups (q/k replicated 4x by column-packed projections).  Score PSUM
    tiles are 2 banks with bufs=2 so pack matmuls double-buffer against
    the ACT exp (FD 1024) -- exps run back-to-back.
  - per-super tiles for xb/xkv/q4/k4 keep DMA/compute deps precise.
  - all matmuls bf16; residual add in fp32 against fp32 PSUM.
"""

import ml_dtypes
import numpy as np

import concourse.bacc as bacc
import concourse.mybir as mybir
from concourse.tile import TileContext
from concourse.bass_utils import run_bass_kernel_spmd

B, C, H, W = 8, 256, 64, 64
N = H * W          # 4096
D = 32             # q/k dim
NK = (H // 2) * (W // 2)   # 1024
NCORES = 8

F32 = mybir.dt.float32
BF16 = mybir.dt.bfloat16

SCALE = 1.0 / np.sqrt(np.float32(D))   # softmax scale

SUP = 1024          # n-super width
NSUP = N // SUP     # 4
MT = NK // 128      # 8 m-tiles
VW = 257            # aggregation rhs width: C channels + ones column


def build_nc():
    nc = bacc.Bacc(None, target_bir_lowering=False, debug=False)

    xb_in = nc.dram_tensor("xb", [C, N], BF16, kind="ExternalInput")
    xt_in = nc.dram_tensor("xt", [N, C], F32, kind="ExternalInput")
    WPACK = D + D + C   # wqT | wkT | wovT along the free dim
    wall_in = nc.dram_tensor("wall", [C, WPACK], BF16, kind="ExternalInput")
    y_out = nc.dram_tensor("y", [N, C], F32, kind="ExternalOutput")
    ytv = y_out.rearrange("(s t p) c -> p s t c", p=128, t=MT)
    xbv = xb_in.rearrange("(t p) n -> p t n", p=128)
    xtv = xt_in.rearrange("(s t p) c -> p s t c", p=128, t=MT)

    with TileContext(nc) as tc:
        with (
            tc.tile_pool(name="big", bufs=1) as big,
            tc.tile_pool(name="etp", bufs=2) as etp,
            tc.tile_pool(name="xres", bufs=2) as xpool,
            tc.tile_pool(name="ystage", bufs=2) as ypool,
            tc.tile_pool(name="small", bufs=8) as smallp,
            tc.tile_pool(name="ps_sc", bufs=2, space="PSUM") as ps_sc,
            tc.tile_pool(name="ps_av", bufs=4, space="PSUM") as ps_av,
        ):
            # ---------------- persistent SBUF ----------------
            xb_t = [
                big.tile([128, 2, SUP], BF16, tag=f"xb{s}", name=f"xb{s}")
                for s in range(NSUP)
            ]
            xkv_t = [
                big.tile([128, 2, 256], BF16, tag=f"xkv{s}", name=f"xkv{s}")
                for s in range(NSUP)
            ]
            q4_t = [
                big.tile([128, SUP], BF16, tag=f"q4{s}", name=f"q4{s}")
                for s in range(NSUP)
            ]
            k4_t = [
                big.tile([128, 256], BF16, tag=f"k4{s}", name=f"k4{s}")
                for s in range(NSUP)
            ]
            vT_sb = big.tile([128, MT, VW], BF16, tag="vT")   # [v^T | ones]
            # DMA staging for weights; the ACT copy into the real tile makes
            # every matmul weight-dependency an ACT-sem wait.
            w_st = big.tile([128, 2, WPACK], BF16, tag="w_st")
            w_sb = big.tile([128, 2, WPACK], BF16, tag="w_sb")
            wq_sb = w_sb[:, :, 0:D]
            wk_sb = w_sb[:, :, D : 2 * D]
            wv_sb = w_sb[:, :, 2 * D :]

            # ---------------- input DMAs ----------------
            nc.sync.dma_start(
                out=w_st, in_=wall_in.rearrange("(t p) w -> p t w", p=128)
            )
            for s in range(NSUP):
                nsl = slice(s * SUP, (s + 1) * SUP)
                nc.sync.dma_start(out=xb_t[s], in_=xbv[:, :, nsl])
            nc.scalar.activation(
                out=w_sb, in_=w_st, func=mybir.ActivationFunctionType.Copy
            )
            # ones column of the aggregation rhs
            nc.vector.memset(vT_sb[:, :, C : C + 1], 1.0)
            # dummy exp: pulls the ACT exp table load into the setup phase
            warm = smallp.tile([128, 1], F32, tag="warm")
            nc.vector.memset(warm, 0.0)
            nc.scalar.activation(
                out=warm, in_=warm, func=mybir.ActivationFunctionType.Exp
            )
            # HAM warm-up: dummy matmuls fill the PE-idle DMA-wait window
            wrm_sb = smallp.tile([128, 512], BF16, tag="wrm")
            nc.vector.memset(wrm_sb, 0.0)
            wrm_ps = ps_sc.tile([128, 512], F32, tag="sc", name="wrm_ps")
            for i in range(8):
                nc.tensor.matmul(
                    wrm_ps, lhsT=wrm_sb[:, 0:128], rhs=wrm_sb,
                    start=(i == 0), stop=(i == 7),
                )

            # -------- prologue: per-super q-proj / avgpool / k / v ---------
            for s in range(NSUP):
                # q projection, 4x column-packed (replicated on 4 groups)
                qp = ps_sc.tile([128, SUP], F32, tag="sc", name="qp")
                for j in range(4):
                    for half in range(2):
                        hsl = slice(half * 512, (half + 1) * 512)
                        for ch in range(2):
                            nc.tensor.matmul(
                                qp[32 * j : 32 * (j + 1), hsl],
                                lhsT=wq_sb[:, ch, :],
                                rhs=xb_t[s][:, ch, hsl],
                                start=(ch == 0),
                                stop=(ch == 1),
                                tile_position=(0, 32 * j),
                            )
                nc.scalar.activation(
                    out=q4_t[s], in_=qp,
                    func=mybir.ActivationFunctionType.Copy,
                )

                # avgpool (sum; /4 folded into wkT/wovT on host)
                x4 = xb_t[s].rearrange("p c (h w t) -> p c h w t", h=16, w=32)
                for ch in range(2):
                    xw = smallp.tile([128, 16, 32], BF16, tag="xw")
                    nc.vector.tensor_add(
                        out=xw, in0=x4[:, ch, :, :, 0], in1=x4[:, ch, :, :, 1]
                    )
                    xh = xw.rearrange("p (h2 t) w -> p h2 t w", t=2)
                    xkv_v = xkv_t[s][:, ch, :].rearrange("p (a b) -> p a b", a=8)
                    nc.vector.tensor_add(
                        out=xkv_v, in0=xh[:, :, 0, :], in1=xh[:, :, 1, :]
                    )

                # k projection for this m-chunk, 4x column-packed
                kp = ps_av.tile([128, 256], F32, tag="av", name="kp")
                for j in range(4):
                    for ch in range(2):
                        nc.tensor.matmul(
                            kp[32 * j : 32 * (j + 1), :],
                            lhsT=wk_sb[:, ch, :],
                            rhs=xkv_t[s][:, ch, :],
                            start=(ch == 0),
                            stop=(ch == 1),
                            tile_position=(0, 32 * j),
                        )
                nc.vector.tensor_copy(out=k4_t[s], in_=kp)

                # v projection (transposed, wov folded) for 2 m-tiles
                for mi in range(2):
                    mt = 2 * s + mi
                    vp = ps_av.tile([128, C], F32, tag="av", name="vp")
                    for ch in range(2):
                        nc.tensor.matmul(
                            vp,
                            lhsT=xkv_t[s][:, ch, mi * 128 : (mi + 1) * 128],
                            rhs=wv_sb[:, ch, :],
                            start=(ch == 0),
                            stop=(ch == 1),
                        )
                    nc.vector.tensor_copy(out=vT_sb[:, mt, 0:C], in_=vp)

            # residual prefetch for super 0 -- on the sync queue so it
            # issues AFTER the xb loads (FIFO per queue) and doesn't steal
            # HBM bandwidth from the prologue-critical xb DMAs
            xt_t = {0: xpool.tile([128, MT, C], F32, tag="xt", name="xt0")}
            nc.sync.dma_start(out=xt_t[0], in_=xtv[:, 0])

            # ---------------- main loop over n-supers ----------------
            def make_packs(s):
                """8 closures; pack(mt) = 2 row-group-concurrent score
                matmuls (one per 512-half) + 1 exp over [128, 1024]."""
                et = etp.tile([128, MT, SUP], BF16, tag="et", name="et")

                def pack(mt):
                    sc = ps_sc.tile([128, 2, 512], F32, tag="sc", name="sc")
                    msl = slice((mt % 2) * 128, (mt % 2 + 1) * 128)
                    for half in range(2):
                        i = 2 * (mt % 2) + half   # row group
                        base = slice(32 * i, 32 * (i + 1))
                        nc.tensor.matmul(
                            sc[:, half, :],
                            lhsT=k4_t[mt // 2][base, msl],
                            rhs=q4_t[s][base, half * 512 : (half + 1) * 512],
                            tile_position=(32 * i, 0),
                        )
                    nc.scalar.activation(
                        out=et[:, mt, :],
                        in_=sc,
                        func=mybir.ActivationFunctionType.Exp,
                        scale=float(SCALE),
                    )

                return et, [lambda mt=mt: pack(mt) for mt in range(MT)]

            cur = make_packs(0)
            for op in cur[1]:
                op()
            for s in range(NSUP):
                et, _ = cur
                nxt_packs = []
                if s + 1 < NSUP:
                    cur = make_packs(s + 1)
                    nxt_packs = list(cur[1])
                    xt_t[s + 1] = xpool.tile(
                        [128, MT, C], F32, tag="xt", name="xtn"
                    )
                    nc.gpsimd.dma_start(out=xt_t[s + 1], in_=xtv[:, s + 1])

                def next_pack():
                    if nxt_packs:
                        nxt_packs.pop(0)()

                y_sb = ypool.tile([128, MT, C], F32, tag="y")
                for grp in range(4):
                    av = {}
                    for mt in range(MT):
                        if mt in (0, 4):
                            next_pack()
                        for t in (2 * grp, 2 * grp + 1):
                            if mt == 0:
                                av[t] = ps_av.tile(
                                    [128, VW], F32, tag="av", name=f"av{t % 2}"
                                )
                            nc.tensor.matmul(
                                av[t],
                                lhsT=et[:, mt, t * 128 : (t + 1) * 128],
                                rhs=vT_sb[:, mt, :],
                                start=(mt == 0),
                                stop=(mt == MT - 1),
                            )
                    for t in (2 * grp, 2 * grp + 1):
                        rc = smallp.tile([128, 1], F32, tag="rc")
                        nc.vector.reciprocal(out=rc, in_=av[t][:, C : C + 1])
                        nc.vector.scalar_tensor_tensor(
                            out=y_sb[:, t, :],
                            in0=av[t][:, 0:C],
                            scalar=rc,
                            in1=xt_t[s][:, t, :],
                            op0=mybir.AluOpType.mult,
                            op1=mybir.AluOpType.add,
                        )
                    # finer stores on the last super shrink the DMA tail
                    if s == NSUP - 1:
                        nc.sync.dma_start(
                            out=ytv[:, s, 2 * grp : 2 * grp + 2],
                            in_=y_sb[:, 2 * grp : 2 * grp + 2],
                        )
                    elif grp % 2 == 1:
                        nc.sync.dma_start(
                            out=ytv[:, s, 2 * grp - 2 : 2 * grp + 2],
                            in_=y_sb[:, 2 * grp - 2 : 2 * grp + 2],
                        )
                while nxt_packs:
                    next_pack()
    nc.compile()
    return nc


_NC_CACHE = {}


def _get_nc():
    if "nc" not in _NC_CACHE:
        _NC_CACHE["nc"] = build_nc()
    return _NC_CACHE["nc"]


def _prep_inputs(x, wq, wk, wv, wo, gamma):
    bf = ml_dtypes.bfloat16
    x = np.asarray(x, dtype=np.float32).reshape(B, C, N)
    g = np.float32(np.asarray(gamma, np.float32)[0])
    wqT = np.asarray(wq, np.float32).T
    wkT = np.asarray(wk, np.float32).T * 0.25
    wov = (0.25 * g) * (np.asarray(wo, np.float32) @ np.asarray(wv, np.float32))
    wall = np.ascontiguousarray(
        np.concatenate([wqT, wkT, wov.T], axis=1)
    ).astype(bf)
    in_maps = []
    for i in range(NCORES):
        xi = x[i]
        in_maps.append({
            "xb": np.ascontiguousarray(xi).astype(bf),
            "xt": np.ascontiguousarray(xi.T),
            "wall": wall,
        })
    return in_maps


def run(x, wq, wk, wv, wo, gamma, trace=False, **trace_kwargs):
    nc = _get_nc()
    in_maps = _prep_inputs(x, wq, wk, wv, wo, gamma)
    res = run_bass_kernel_spmd(
        nc, in_maps, list(range(NCORES)), trace=trace, **trace_kwargs
    )
    y = np.stack([
        np.ascontiguousarray(res.results[i]["y"].T).reshape(C, H, W)
        for i in range(NCORES)
    ])
    return y, res


def kernel(x, wq, wk, wv, wo, gamma):
    y, _ = run(x, wq, wk, wv, wo, gamma, trace=False)
    return y


# revision 10
# speedup vs baseline: 1.2650x; 1.0786x over previous
"""Trainium2 Bass kernel for BasicSelfAttention2D (spatial-reduction attention).

Reference computation (per image):
    q   = (wq @ x_flat)              [d=32, N=4096]
    xkv = avgpool2x2(x)              [C, Nk=1024]
    k   = wk @ xkv                   [d, Nk]
    v   = wv @ xkv                   [C, Nk]
    attn= softmax(q^T k / sqrt(d))   [N, Nk]
    out = v @ attn^T                 [C, N]
    y   = x + gamma * (wo @ out)

Sharding: data-parallel over batch, one image per NeuronCore (8 cores).

Kernel strategy (per core):
  - HOST folds wov = 0.25*gamma*(wo @ wv): the aggregation matmul then
    directly produces the final (pre-residual) output -- no separate
    out-projection pass on the PE.
  - TRANSPOSED aggregation: out_T[n, c] = sum_m et[m, n] * vT[m, c] with
    lhsT = et (m on partitions) and rhs = [vT | ones].  The appended ones
    column makes column C of the PSUM output the softmax denominator
    rsum[n], which lands per-partition -- so softmax scale + residual add
    fuse into ONE scalar_tensor_tensor per 128-row n-tile:
        y_T[n, :] = av[n, :] * recip(rsum[n]) + x_T[n, :]
    x is loaded (and y stored) in transposed [N, C] layout; the host
    transposes y back (cheap numpy work, not HW time).
  - scores are built transposed s_T[m, n]: one pack per m-tile covers both
    512-halves of the n-super concurrently on two distinct 32-row PE
    groups (q/k replicated 4x by column-packed projections).  Score PSUM
    tiles are 2 banks with bufs=2 so pack matmuls double-buffer against
    the ACT exp (FD 1024) -- exps run back-to-back.
  - HAM discipline: the PE clock gate re-throttles to 1.2 GHz after ~3.4us
    of idle, and a mid-kernel re-throttle costs ~2x on every matmul until
    it recovers.  So: xb loads go out on two DMA queues in parallel, warm
    matmuls bridge the load window, and super-0 score packs are emitted
    inside the prologue so the PE never starves early.
  - all matmuls bf16; residual add in fp32 against fp32 PSUM.
"""

import ml_dtypes
import numpy as np

import concourse.bacc as bacc
import concourse.mybir as mybir
from concourse.tile import TileContext
from concourse.bass_utils import run_bass_kernel_spmd

B, C, H, W = 8, 256, 64, 64
N = H * W          # 4096
D = 32             # q/k dim
NK = (H // 2) * (W // 2)   # 1024
NCORES = 8

F32 = mybir.dt.float32
BF16 = mybir.dt.bfloat16

SCALE = 1.0 / np.sqrt(np.float32(D))   # softmax scale

SUP = 1024          # n-super width
NSUP = N // SUP     # 4
MT = NK // 128      # 8 m-tiles
VW = 257            # aggregation rhs width: C channels + ones column


def build_nc():
    nc = bacc.Bacc(None, target_bir_lowering=False, debug=False)

    xb_in = nc.dram_tensor("xb", [C, N], BF16, kind="ExternalInput")
    xt_in = nc.dram_tensor("xt", [N, C], F32, kind="ExternalInput")
    WPACK = D + D + C   # wqT | wkT | wovT along the free dim
    wall_in = nc.dram_tensor("wall", [C, WPACK], BF16, kind="ExternalInput")
    y_out = nc.dram_tensor("y", [N, C], F32, kind="ExternalOutput")
    ytv = y_out.rearrange("(s t p) c -> p s t c", p=128, t=MT)
    xbv = xb_in.rearrange("(t p) n -> p t n", p=128)
    xtv = xt_in.rearrange("(s t p) c -> p s t c", p=128, t=MT)

    with TileContext(nc) as tc:
        with (
            tc.tile_pool(name="big", bufs=1) as big,
            tc.tile_pool(name="etp", bufs=2) as etp,
            tc.tile_pool(name="xres", bufs=2) as xpool,
            tc.tile_pool(name="ystage", bufs=2) as ypool,
            tc.tile_pool(name="small", bufs=8) as smallp,
            tc.tile_pool(name="ps_sc", bufs=2, space="PSUM") as ps_sc,
            tc.tile_pool(name="ps_av", bufs=4, space="PSUM") as ps_av,
        ):
            # ---------------- persistent SBUF ----------------
            xb_t = [
                big.tile([128, 2, SUP], BF16, tag=f"xb{s}", name=f"xb{s}")
                for s in range(NSUP)
            ]
            xkv_t = [
                big.tile([128, 2, 256], BF16, tag=f"xkv{s}", name=f"xkv{s}")
                for s in range(NSUP)
            ]
            q4_t = [
                big.tile([128, SUP], BF16, tag=f"q4{s}", name=f"q4{s}")
                for s in range(NSUP)
            ]
            k4_t = [
                big.tile([128, 256], BF16, tag=f"k4{s}", name=f"k4{s}")
                for s in range(NSUP)
            ]
            vT_sb = big.tile([128, MT, VW], BF16, tag="vT")   # [v^T | ones]
            # DMA staging for weights; the ACT copy into the real tile makes
            # every matmul weight-dependency an ACT-sem wait.
            w_st = big.tile([128, 2, WPACK], BF16, tag="w_st")
            w_sb = big.tile([128, 2, WPACK], BF16, tag="w_sb")
            wq_sb = w_sb[:, :, 0:D]
            wk_sb = w_sb[:, :, D : 2 * D]
            wv_sb = w_sb[:, :, 2 * D :]

            # ---------------- input DMAs (two HWDGE queues) ----------------
            nc.scalar.dma_start(out=xb_t[2], in_=xbv[:, :, 2 * SUP : 3 * SUP])
            nc.scalar.dma_start(out=xb_t[3], in_=xbv[:, :, 3 * SUP : 4 * SUP])
            nc.sync.dma_start(
                out=w_st, in_=wall_in.rearrange("(t p) w -> p t w", p=128)
            )
            nc.sync.dma_start(out=xb_t[0], in_=xbv[:, :, 0:SUP])
            nc.sync.dma_start(out=xb_t[1], in_=xbv[:, :, SUP : 2 * SUP])
            nc.scalar.activation(
                out=w_sb, in_=w_st, func=mybir.ActivationFunctionType.Copy
            )
            # ones column of the aggregation rhs
            nc.vector.memset(vT_sb[:, :, C : C + 1], 1.0)
            # dummy exp: pulls the ACT exp table load into the setup phase
            warm = smallp.tile([128, 1], F32, tag="warm")
            nc.vector.memset(warm, 0.0)
            nc.scalar.activation(
                out=warm, in_=warm, func=mybir.ActivationFunctionType.Exp
            )
            # HAM warm-up: dummy matmuls bridge the PE-idle DMA-wait window
            # (a >3.4us PE gap re-throttles the clock gate to 1.2 GHz)
            wrm_sb = smallp.tile([128, 512], BF16, tag="wrm")
            nc.vector.memset(wrm_sb, 0.0)
            wrm_ps = ps_sc.tile([128, 2, 512], F32, tag="sc", name="wrm_ps")
            for i in range(12):
                nc.tensor.matmul(
                    wrm_ps[:, i % 2, :], lhsT=wrm_sb[:, 0:128], rhs=wrm_sb,
                    start=(i < 2), stop=(i >= 10),
                )

            # residual prefetch for super 0 -- behind the xb loads on the
            # sync queue so it doesn't steal HBM bandwidth from them
            xt_t = {0: xpool.tile([128, MT, C], F32, tag="xt", name="xt0")}
            nc.sync.dma_start(out=xt_t[0], in_=xtv[:, 0])

            et_t = {0: etp.tile([128, MT, SUP], BF16, tag="et", name="et0")}

            def emit_pack(s, mt):
                """One score pack: 2 row-group-concurrent K=32 matmuls (one
                per 512-half of the n-super) + 1 exp over [128, 1024]."""
                sc = ps_sc.tile([128, 2, 512], F32, tag="sc", name="sc")
                msl = slice((mt % 2) * 128, (mt % 2 + 1) * 128)
                for half in range(2):
                    i = 2 * (mt % 2) + half   # row group
                    base = slice(32 * i, 32 * (i + 1))
                    nc.tensor.matmul(
                        sc[:, half, :],
                        lhsT=k4_t[mt // 2][base, msl],
                        rhs=q4_t[s][base, half * 512 : (half + 1) * 512],
                        tile_position=(32 * i, 0),
                    )
                nc.scalar.activation(
                    out=et_t[s][:, mt, :],
                    in_=sc,
                    func=mybir.ActivationFunctionType.Exp,
                    scale=float(SCALE),
                )

            # -------- prologue: per-super q-proj / avgpool / k / v ---------
            # (+ super-0 score packs, so PE work is continuous into the
            #  main loop and ACT starts retiring super-0 exps early)
            for s in range(NSUP):
                # q projection, 4x column-packed (replicated on 4 groups),
                # in 512-wide chunks through 1-bank PSUM tiles
                for cc in range(2):
                    csl = slice(cc * 512, (cc + 1) * 512)
                    qp = ps_av.tile([128, 512], F32, tag="av", name="qp")
                    for j in range(4):
                        for ch in range(2):
                            nc.tensor.matmul(
                                qp[32 * j : 32 * (j + 1), :],
                                lhsT=wq_sb[:, ch, :],
                                rhs=xb_t[s][:, ch, csl],
                                start=(ch == 0),
                                stop=(ch == 1),
                                tile_position=(0, 32 * j),
                            )
                    nc.scalar.activation(
                        out=q4_t[s][:, csl], in_=qp,
                        func=mybir.ActivationFunctionType.Copy,
                    )

                # avgpool (sum; /4 folded into wkT/wovT on host)
                x4 = xb_t[s].rearrange("p c (h w t) -> p c h w t", h=16, w=32)
                for ch in range(2):
                    xw = smallp.tile([128, 16, 32], BF16, tag="xw")
                    nc.vector.tensor_add(
                        out=xw, in0=x4[:, ch, :, :, 0], in1=x4[:, ch, :, :, 1]
                    )
                    xh = xw.rearrange("p (h2 t) w -> p h2 t w", t=2)
                    xkv_v = xkv_t[s][:, ch, :].rearrange("p (a b) -> p a b", a=8)
                    nc.vector.tensor_add(
                        out=xkv_v, in0=xh[:, :, 0, :], in1=xh[:, :, 1, :]
                    )

                # k projection for this m-chunk, 4x column-packed
                kp = ps_av.tile([128, 256], F32, tag="av", name="kp")
                for j in range(4):
                    for ch in range(2):
                        nc.tensor.matmul(
                            kp[32 * j : 32 * (j + 1), :],
                            lhsT=wk_sb[:, ch, :],
                            rhs=xkv_t[s][:, ch, :],
                            start=(ch == 0),
                            stop=(ch == 1),
                            tile_position=(0, 32 * j),
                        )
                nc.vector.tensor_copy(out=k4_t[s], in_=kp)

                # v projection (transposed, wov folded) for 2 m-tiles
                for mi in range(2):
                    mt = 2 * s + mi
                    vp = ps_av.tile([128, C], F32, tag="av", name="vp")
                    for ch in range(2):
                        nc.tensor.matmul(
                            vp,
                            lhsT=xkv_t[s][:, ch, mi * 128 : (mi + 1) * 128],
                            rhs=wv_sb[:, ch, :],
                            start=(ch == 0),
                            stop=(ch == 1),
                        )
                    nc.vector.tensor_copy(out=vT_sb[:, mt, 0:C], in_=vp)

                # super-0 score packs for the two m-tiles just produced
                emit_pack(0, 2 * s)
                emit_pack(0, 2 * s + 1)

            # ---------------- main loop over n-supers ----------------
            for s in range(NSUP):
                et = et_t[s]
                nxt_packs = []
                if s + 1 < NSUP:
                    et_t[s + 1] = etp.tile(
                        [128, MT, SUP], BF16, tag="et", name="etn"
                    )
                    nxt_packs = [
                        (lambda mt=mt: emit_pack(s + 1, mt)) for mt in range(MT)
                    ]
                    xt_t[s + 1] = xpool.tile(
                        [128, MT, C], F32, tag="xt", name="xtn"
                    )
                    nc.gpsimd.dma_start(out=xt_t[s + 1], in_=xtv[:, s + 1])

                def next_pack():
                    if nxt_packs:
                        nxt_packs.pop(0)()

                y_sb = ypool.tile([128, MT, C], F32, tag="y")
                for grp in range(4):
                    av = {}
                    for mt in range(MT):
                        if mt in (0, 4):
                            next_pack()
                        for t in (2 * grp, 2 * grp + 1):
                            if mt == 0:
                                av[t] = ps_av.tile(
                                    [128, VW], F32, tag="av", name=f"av{t % 2}"
                                )
                            nc.tensor.matmul(
                                av[t],
                                lhsT=et[:, mt, t * 128 : (t + 1) * 128],
                                rhs=vT_sb[:, mt, :],
                                start=(mt == 0),
                                stop=(mt == MT - 1),
                            )
                    for t in (2 * grp, 2 * grp + 1):
                        rc = smallp.tile([128, 1], F32, tag="rc")
                        nc.vector.reciprocal(out=rc, in_=av[t][:, C : C + 1])
                        nc.vector.scalar_tensor_tensor(
                            out=y_sb[:, t, :],
                            in0=av[t][:, 0:C],
                            scalar=rc,
                            in1=xt_t[s][:, t, :],
                            op0=mybir.AluOpType.mult,
                            op1=mybir.AluOpType.add,
                        )
                    # finer stores on the last super shrink the DMA tail
                    if s == NSUP - 1:
                        nc.sync.dma_start(
                            out=ytv[:, s, 2 * grp : 2 * grp + 2],
                            in_=y_sb[:, 2 * grp : 2 * grp + 2],
                        )
                    elif grp % 2 == 1:
                        nc.sync.dma_start(
                            out=ytv[:, s, 2 * grp - 2 : 2 * grp + 2],
                            in_=y_sb[:, 2 * grp - 2 : 2 * grp + 2],
                        )
                while nxt_packs:
                    next_pack()
    nc.compile()
    return nc


_NC_CACHE = {}


def _get_nc():
    if "nc" not in _NC_CACHE:
        _NC_CACHE["nc"] = build_nc()
    return _NC_CACHE["nc"]


def _prep_inputs(x, wq, wk, wv, wo, gamma):
    bf = ml_dtypes.bfloat16
    x = np.asarray(x, dtype=np.float32).reshape(B, C, N)
    g = np.float32(np.asarray(gamma, np.float32)[0])
    wqT = np.asarray(wq, np.float32).T
    wkT = np.asarray(wk, np.float32).T * 0.25
    wov = (0.25 * g) * (np.asarray(wo, np.float32) @ np.asarray(wv, np.float32))
    wall = np.ascontiguousarray(
        np.concatenate([wqT, wkT, wov.T], axis=1)
    ).astype(bf)
    in_maps = []
    for i in range(NCORES):
        xi = x[i]
        in_maps.append({
            "xb": np.ascontiguousarray(xi).astype(bf),
            "xt": np.ascontiguousarray(xi.T),
            "wall": wall,
        })
    return in_maps


def run(x, wq, wk, wv, wo, gamma, trace=False, **trace_kwargs):
    nc = _get_nc()
    in_maps = _prep_inputs(x, wq, wk, wv, wo, gamma)
    res = run_bass_kernel_spmd(
        nc, in_maps, list(range(NCORES)), trace=trace, **trace_kwargs
    )
    y = np.stack([
        np.ascontiguousarray(res.results[i]["y"].T).reshape(C, H, W)
        for i in range(NCORES)
    ])
    return y, res


def kernel(x, wq, wk, wv, wo, gamma):
    y, _ = run(x, wq, wk, wv, wo, gamma, trace=False)
    return y
